# revision 6
# baseline (speedup 1.0000x reference)
"""Trainium2 Bass kernel for a dense 6-expert "constraint MoE".

Reference computation (f32), per expert e in 0..5:
    x1      = act_e(h @ W1[e] + b1[e])        # [BT, H]
    x2      = act_e(x1 @ W2[e] + b2[e])       # [BT, H]
    main   += x2 @ Wm[e] + bm[e]              # [BT, D]
    aux[e]  = x2 @ Wc[e] + bc[e]              # [BT, OUT]

Sharding: data-parallel over the B*T = 8192 tokens across 8 NeuronCores
(1024 tokens per core); the dense expert weights are replicated. On chip
every activation is kept feature-major ([feature, token]) so each weight
matrix loads untransposed as the matmul stationary operand and no on-chip
transposes are needed; the host transposes h once on the way in and the
outputs once on the way out. Matmuls run in bf16 with f32 PSUM
accumulation; the cross-expert `main` sum is accumulated f32 in SBUF by
the vector engine.
"""

import sys
import time
from contextlib import ExitStack

import numpy as np
import ml_dtypes

if "/opt/trn_rl_repo" not in sys.path:
    sys.path.insert(0, "/opt/trn_rl_repo")

import concourse.bacc as bacc
import concourse.bass as bass
import concourse.mybir as mybir
import concourse.tile as tile

AF = mybir.ActivationFunctionType

# Problem sizes (hardcoded; must match the grader's setup_inputs()).
E, D, H, OUT = 6, 1024, 4096, 4
B, T = 4, 2048
N_CORES = 8
TOK = (B * T) // N_CORES  # tokens per core

# jax.nn.gelu default is approximate=True -> tanh approximation.
ACT_FNS = [AF.Relu, AF.Gelu_apprx_tanh, AF.Tanh, AF.Silu, AF.Relu, AF.Gelu_apprx_tanh]

BF16 = mybir.dt.bfloat16
F32 = mybir.dt.float32
NPBF16 = ml_dtypes.bfloat16

NSUB = 512  # matmul moving free dim / PSUM bank width in f32


def build_nc(e_=E, d_=D, h_=H, out_=OUT, tok_=TOK, act_fns=None):
    """Build the per-core Bass program (identical on all cores; SPMD)."""
    act_fns = act_fns if act_fns is not None else ACT_FNS
    KD, KH = d_ // 128, h_ // 128  # k/f tile counts for D and H
    S = tok_ // NSUB  # token sub-tiles per matmul group
    assert tok_ % NSUB == 0 and d_ % 128 == 0 and h_ % 128 == 0

    nc = bacc.Bacc(None, target_bir_lowering=False)

    xT = nc.dram_tensor("xT", [d_, tok_], BF16, kind="ExternalInput")
    W1 = nc.dram_tensor("W1", [e_, d_, h_], BF16, kind="ExternalInput")
    W2 = nc.dram_tensor("W2", [e_, h_, h_], BF16, kind="ExternalInput")
    Wm = nc.dram_tensor("Wm", [e_, h_, d_], BF16, kind="ExternalInput")
    Wc = nc.dram_tensor("Wc", [e_, h_, out_], BF16, kind="ExternalInput")
    b1T = nc.dram_tensor("b1T", [e_, 128, KH], F32, kind="ExternalInput")
    b2T = nc.dram_tensor("b2T", [e_, 128, KH], F32, kind="ExternalInput")
    bmsT = nc.dram_tensor("bmsT", [128, KD], F32, kind="ExternalInput")  # sum_e bm[e]
    bc = nc.dram_tensor("bc", [e_, out_, 1], F32, kind="ExternalInput")
    mainT = nc.dram_tensor("mainT", [d_, tok_], F32, kind="ExternalOutput")
    auxT = nc.dram_tensor("auxT", [e_, out_, tok_], F32, kind="ExternalOutput")

    C = 2  # f-tiles per PSUM chunk (2 tiles x 2 banks = 4 banks in flight)
    KG = 4  # k-tiles per weight-slab DMA

    with tile.TileContext(nc) as tc, ExitStack() as ctx:
        xp = ctx.enter_context(tc.tile_pool(name="xp", bufs=KD))
        t1p = ctx.enter_context(tc.tile_pool(name="t1p", bufs=KH))
        t2p = ctx.enter_context(tc.tile_pool(name="t2p", bufs=KH))
        mp = ctx.enter_context(tc.tile_pool(name="mp", bufs=KD))
        wp = ctx.enter_context(tc.tile_pool(name="wp", bufs=3))
        wcp = ctx.enter_context(tc.tile_pool(name="wcp", bufs=2))
        bp = ctx.enter_context(tc.tile_pool(name="bp", bufs=2))
        ap = ctx.enter_context(tc.tile_pool(name="ap", bufs=2))
        psp = ctx.enter_context(
            tc.tile_pool(name="psp", bufs=3, space=bass.MemorySpace.PSUM)
        )
        pxp = ctx.enter_context(
            tc.tile_pool(name="pxp", bufs=1, space=bass.MemorySpace.PSUM)
        )

        # Input activations, feature-major, resident for the whole kernel.
        xt = []
        for k in range(KD):
            t = xp.tile([128, tok_], BF16, tag="x", name=f"x{k}")
            nc.sync.dma_start(t[:], xT[k * 128 : (k + 1) * 128, :])
            xt.append(t)

        # f32 accumulator for main, resident.
        macc = [mp.tile([128, tok_], F32, tag="macc", name=f"macc{d}") for d in range(KD)]

        def linear_phase(src, wd, ft, kt, out_cb, tag):
            """out[f] = sum_k wd[k-tile, f-tile].T @ src[k], f in 0..ft.

            wd is the DRAM weight view [kt*128, ft*128]. PSUM chunks of C
            f-tiles; weight slabs of KG k-tiles x C*128 columns stream
            through `wp`. out_cb(f, ps) consumes the accumulated [128,S,512]
            PSUM tile.
            """
            for c0 in range(0, ft, C):
                cf = min(C, ft - c0)
                ps = [
                    psp.tile([128, S, NSUB], F32, tag="ps", name=f"ps_{tag}_{c0}_{i}")
                    for i in range(cf)
                ]
                for kb in range(0, kt, KG):
                    kg = min(KG, kt - kb)
                    wt = wp.tile([128, kg, cf * 128], BF16, tag="w", name=f"w_{tag}_{c0}_{kb}")
                    nc.sync.dma_start(
                        wt[:],
                        wd[
                            kb * 128 : (kb + kg) * 128, c0 * 128 : (c0 + cf) * 128
                        ].rearrange("(kk p) j -> p kk j", p=128),
                    )
                    for kk in range(kg):
                        k = kb + kk
                        for i in range(cf):
                            for s in range(S):
                                nc.tensor.matmul(
                                    ps[i][:, s, :],
                                    wt[:, kk, i * 128 : (i + 1) * 128],
                                    src[k][:, s * NSUB : (s + 1) * NSUB],
                                    start=(k == 0),
                                    stop=(k == kt - 1),
                                )
                for i in range(cf):
                    out_cb(c0 + i, ps[i])

        for e in range(e_):
            act = act_fns[e]

            b1t = bp.tile([128, KH], F32, tag="b1", name=f"b1_{e}")
            nc.sync.dma_start(b1t[:], b1T[e])
            b2t = bp.tile([128, KH], F32, tag="b2", name=f"b2_{e}")
            nc.sync.dma_start(b2t[:], b2T[e])
            bct = bp.tile([out_, 1], F32, tag="bc", name=f"bc_{e}")
            nc.sync.dma_start(bct[:], bc[e])

            # ---- Phase 1: t1 = act(x @ W1 + b1), feature-major bf16
            t1 = []

            def p1_out(f, ps, e=e, act=act, b1t=b1t, t1=t1):
                tt = t1p.tile([128, tok_], BF16, tag="t1", name=f"t1_{e}_{f}")
                for s in range(S):
                    nc.scalar.activation(
                        tt[:, s * NSUB : (s + 1) * NSUB],
                        ps[:, s, :],
                        act,
                        bias=b1t[:, f : f + 1],
                    )
                t1.append(tt)

            linear_phase(xt, W1[e], KH, KD, p1_out, f"p1e{e}")

            # ---- Phase 2: t2 = act(t1 @ W2 + b2)
            t2 = []

            def p2_out(f, ps, e=e, act=act, b2t=b2t, t2=t2):
                tt = t2p.tile([128, tok_], BF16, tag="t2", name=f"t2_{e}_{f}")
                for s in range(S):
                    nc.scalar.activation(
                        tt[:, s * NSUB : (s + 1) * NSUB],
                        ps[:, s, :],
                        act,
                        bias=b2t[:, f : f + 1],
                    )
                t2.append(tt)

            linear_phase(t1, W2[e], KH, KH, p2_out, f"p2e{e}")

            # ---- Phase 3: macc += t2 @ Wm   (bias folded into bmsT at the end)
            def p3_out(dtile, ps, e=e):
                for s in range(S):
                    dst = macc[dtile][:, s * NSUB : (s + 1) * NSUB]
                    if e == 0:
                        nc.vector.tensor_copy(dst, ps[:, s, :])
                    else:
                        nc.vector.tensor_add(dst, dst, ps[:, s, :])

            linear_phase(t2, Wm[e], KD, KH, p3_out, f"p3e{e}")

            # ---- Phase 4: aux[e] = t2 @ Wc + bc
            wct = wcp.tile([128, KH, out_], BF16, tag="wc", name=f"wc_{e}")
            nc.sync.dma_start(wct[:], Wc[e].rearrange("(kk p) j -> p kk j", p=128))
            psa = pxp.tile([out_, S, NSUB], F32, tag="psa", name=f"psa_{e}")
            for k in range(KH):
                for s in range(S):
                    nc.tensor.matmul(
                        psa[:, s, :],
                        wct[:, k, :],
                        t2[k][:, s * NSUB : (s + 1) * NSUB],
                        start=(k == 0),
                        stop=(k == KH - 1),
                    )
            for s in range(S):
                ao = ap.tile([out_, NSUB], F32, tag="ao", name=f"ao_{e}_{s}")
                nc.scalar.activation(ao[:], psa[:, s, :], AF.Identity, bias=bct[:])
                nc.sync.dma_start(auxT[e][:, s * NSUB : (s + 1) * NSUB], ao[:])

        # ---- Epilogue: add summed main bias, store main.
        bmst = bp.tile([128, KD], F32, tag="bms", name="bms")
        nc.sync.dma_start(bmst[:], bmsT[:])
        for dtile in range(KD):
            nc.vector.tensor_scalar_add(
                macc[dtile][:], macc[dtile][:], bmst[:, dtile : dtile + 1]
            )
            nc.sync.dma_start(mainT[dtile * 128 : (dtile + 1) * 128, :], macc[dtile][:])

    nc.compile()
    return nc


class _Exec:
    """Mirror of bass2jax.run_bass_via_pjrt's multi-core path, but keeps the
    jitted executable and the (large, replicated-weight) device inputs alive so
    repeated runs only move the small donated output buffers."""

    def __init__(self, nc, n_cores):
        import jax
        from jax.experimental.shard_map import shard_map
        from jax.sharding import Mesh, NamedSharding, PartitionSpec
        from concourse.bass2jax import (
            _bass_exec_p,
            install_neuronx_cc_hook,
            partition_id_tensor,
        )

        install_neuronx_cc_hook()
        assert nc.dbg_addr is None
        self.jax = jax
        self.nc = nc
        self.n_cores = n_cores
        partition_name = (
            nc.partition_id_tensor.name if nc.partition_id_tensor else None
        )
        in_names, out_names, out_avals, zero_outs = [], [], [], []
        for alloc in nc.m.functions[0].allocations:
            if not isinstance(alloc, mybir.MemoryLocationSet):
                continue
            name = alloc.memorylocations[0].name
            if alloc.kind == "ExternalInput":
                if name != partition_name:
                    in_names.append(name)
            elif alloc.kind == "ExternalOutput":
                out_names.append(name)
                shape = tuple(alloc.tensor_shape)
                dtype = mybir.dt.np(alloc.dtype)
                out_avals.append(jax.core.ShapedArray(shape, dtype))
                zero_outs.append(np.zeros(shape, dtype))
        n_params = len(in_names)
        in_names = in_names + out_names
        if partition_name is not None:
            in_names.append(partition_name)
        self.param_names = in_names[:n_params]
        self.out_names = out_names
        self.out_avals = out_avals
        self.zero_outs = zero_outs
        self.n_params = n_params

        def _body(*args):
            operands = list(args)
            if partition_name is not None:
                operands.append(partition_id_tensor())
            outs = _bass_exec_p.bind(
                *operands,
                out_avals=tuple(out_avals),
                in_names=tuple(in_names),
                out_names=tuple(out_names),
                lowering_input_output_aliases=(),
                sim_require_finite=True,
                sim_require_nnan=True,
                nc=nc,
            )
            return tuple(outs)

        devices = jax.devices()[:n_cores]
        assert len(devices) == n_cores
        self.mesh = Mesh(np.asarray(devices), ("core",))
        self.sharding = NamedSharding(self.mesh, PartitionSpec("core"))
        n_outs = len(out_names)
        donate = tuple(range(n_params, n_params + n_outs))
        in_specs = (PartitionSpec("core"),) * (n_params + n_outs)
        out_specs = (PartitionSpec("core"),) * n_outs
        self.sharded = jax.jit(
            shard_map(
                _body,
                mesh=self.mesh,
                in_specs=in_specs,
                out_specs=out_specs,
                check_rep=False,
            ),
            donate_argnums=donate,
            keep_unused=True,
        )
        self.in_dev = None

    def prepare(self, in_maps):
        """Concat per-core inputs along axis 0 and push to devices."""
        assert len(in_maps) == self.n_cores
        concat_in = [
            np.concatenate([np.asarray(m[name]) for m in in_maps], axis=0)
            for name in self.param_names
        ]
        self.in_dev = [
            self.jax.device_put(a, self.sharding) for a in concat_in
        ]
        self.jax.block_until_ready(self.in_dev)

    def run_once(self):
        """One full execution; returns (per_core_results, exec_seconds).

        The donated output buffers are staged to device before the timed
        region, so exec_seconds ≈ dispatch + NEFF execution."""
        zeros_dev = [
            self.jax.device_put(
                np.zeros((self.n_cores * z.shape[0], *z.shape[1:]), z.dtype),
                self.sharding,
            )
            for z in self.zero_outs
        ]
        self.jax.block_until_ready(zeros_dev)
        t0 = time.perf_counter()
        out_arrs = self.sharded(*self.in_dev, *zeros_dev)
        self.jax.block_until_ready(out_arrs)
        dt = time.perf_counter() - t0
        results = [
            {
                name: np.asarray(out_arrs[i]).reshape(
                    self.n_cores, *self.out_avals[i].shape
                )[c]
                for i, name in enumerate(self.out_names)
            }
            for c in range(self.n_cores)
        ]
        return results, dt


_CACHE = {}


def get_exec():
    if "exec" not in _CACHE:
        _CACHE["exec"] = _Exec(build_nc(), N_CORES)
    return _CACHE["exec"]


def prep_inputs(h, W1, b1, W2, b2, Wm, bm, Wc, bc, n_cores=N_CORES):
    """Host-side shard + layout prep. Returns per-core input maps."""
    e_, d_, h_ = W1.shape
    out_ = Wc.shape[2]
    bt = h.shape[0] * h.shape[1]
    tok = bt // n_cores
    h_flat = np.ascontiguousarray(h.reshape(bt, d_))

    w1b = np.ascontiguousarray(W1.astype(NPBF16))
    w2b = np.ascontiguousarray(W2.astype(NPBF16))
    wmb = np.ascontiguousarray(Wm.astype(NPBF16))
    wcb = np.ascontiguousarray(Wc.astype(NPBF16))
    b1t = np.ascontiguousarray(
        b1.reshape(e_, h_ // 128, 128).transpose(0, 2, 1).astype(np.float32)
    )
    b2t = np.ascontiguousarray(
        b2.reshape(e_, h_ // 128, 128).transpose(0, 2, 1).astype(np.float32)
    )
    bmst = np.ascontiguousarray(
        bm.sum(axis=0).reshape(d_ // 128, 128).T.astype(np.float32)
    )
    bcr = np.ascontiguousarray(bc.reshape(e_, out_, 1).astype(np.float32))

    in_maps = []
    for c in range(n_cores):
        xTc = np.ascontiguousarray(
            h_flat[c * tok : (c + 1) * tok, :].T.astype(NPBF16)
        )
        in_maps.append(
            {
                "xT": xTc,
                "W1": w1b,
                "W2": w2b,
                "Wm": wmb,
                "Wc": wcb,
                "b1T": b1t,
                "b2T": b2t,
                "bmsT": bmst,
                "bc": bcr,
            }
        )
    return in_maps


def assemble_outputs(results, b_, t_, e_, d_, out_):
    """Gather per-core feature-major outputs back to reference layout."""
    main = np.concatenate([np.asarray(r["mainT"]).T for r in results], axis=0)
    main = np.ascontiguousarray(main.reshape(b_, t_, d_).astype(np.float32))
    aux = np.concatenate(
        [np.asarray(r["auxT"]).transpose(0, 2, 1) for r in results], axis=1
    )
    aux = np.ascontiguousarray(aux.reshape(e_, b_, t_, out_).astype(np.float32))
    return main, aux


def run(inputs):
    """Run on hardware; returns ((main, aux), exec_seconds)."""
    ex = get_exec()
    inputs = {k: np.asarray(v) for k, v in inputs.items()}
    in_maps = prep_inputs(**inputs)
    ex.prepare(in_maps)
    results, dt = ex.run_once()
    main, aux = assemble_outputs(results, B, T, E, D, OUT)
    return (main, aux), dt


def kernel(**inputs):
    out, _ = run(inputs)
    return out


# revision 10
# speedup vs baseline: 18.0199x; 18.0199x over previous
"""Trainium2 Bass kernel for a dense 6-expert "constraint MoE".

Reference computation (f32), per expert e in 0..5:
    x1      = act_e(h @ W1[e] + b1[e])        # [BT, H]
    x2      = act_e(x1 @ W2[e] + b2[e])       # [BT, H]
    main   += x2 @ Wm[e] + bm[e]              # [BT, D]
    aux[e]  = x2 @ Wc[e] + bc[e]              # [BT, OUT]

Sharding: data-parallel over the B*T = 8192 tokens across 8 NeuronCores
(1024 tokens per core); the dense expert weights are replicated. On chip
every activation is kept feature-major ([feature, token]) so each weight
matrix loads untransposed as the matmul stationary operand and no on-chip
transposes are needed; the host transposes h once on the way in and the
outputs once on the way out. Matmuls run in bf16 with f32 PSUM
accumulation; the cross-expert `main` sum is accumulated f32 in SBUF by
the vector engine.
"""

import sys
import time
from contextlib import ExitStack

import numpy as np
import ml_dtypes

if "/opt/trn_rl_repo" not in sys.path:
    sys.path.insert(0, "/opt/trn_rl_repo")

import concourse.bacc as bacc
import concourse.bass as bass
import concourse.mybir as mybir
import concourse.tile as tile

AF = mybir.ActivationFunctionType

# Problem sizes (hardcoded; must match the grader's setup_inputs()).
E, D, H, OUT = 6, 1024, 4096, 4
B, T = 4, 2048
N_CORES = 8
TOK = (B * T) // N_CORES  # tokens per core

# jax.nn.gelu default is approximate=True -> tanh approximation.
ACT_FNS = [AF.Relu, AF.Gelu_apprx_tanh, AF.Tanh, AF.Silu, AF.Relu, AF.Gelu_apprx_tanh]

BF16 = mybir.dt.bfloat16
F32 = mybir.dt.float32
NPBF16 = ml_dtypes.bfloat16

NSUB = 512  # matmul moving free dim / PSUM bank width in f32


def build_nc(e_=E, d_=D, h_=H, out_=OUT, tok_=TOK, act_fns=None):
    """Build the per-core Bass program (identical on all cores; SPMD)."""
    act_fns = act_fns if act_fns is not None else ACT_FNS
    KD, KH = d_ // 128, h_ // 128  # k/f tile counts for D and H
    S = tok_ // NSUB  # token sub-tiles per matmul group
    assert tok_ % NSUB == 0 and d_ % 128 == 0 and h_ % 128 == 0

    nc = bacc.Bacc(None, target_bir_lowering=False)

    xT = nc.dram_tensor("xT", [d_, tok_], BF16, kind="ExternalInput")
    W1 = nc.dram_tensor("W1", [e_, d_, h_], BF16, kind="ExternalInput")
    W2 = nc.dram_tensor("W2", [e_, h_, h_], BF16, kind="ExternalInput")
    Wm = nc.dram_tensor("Wm", [e_, h_, d_], BF16, kind="ExternalInput")
    Wc = nc.dram_tensor("Wc", [e_, h_, out_], BF16, kind="ExternalInput")
    b1T = nc.dram_tensor("b1T", [e_, 128, KH], F32, kind="ExternalInput")
    b2T = nc.dram_tensor("b2T", [e_, 128, KH], F32, kind="ExternalInput")
    bmsT = nc.dram_tensor("bmsT", [128, KD], F32, kind="ExternalInput")  # sum_e bm[e]
    bc = nc.dram_tensor("bc", [e_, out_, 1], F32, kind="ExternalInput")
    mainT = nc.dram_tensor("mainT", [d_, tok_], F32, kind="ExternalOutput")
    auxT = nc.dram_tensor("auxT", [e_, out_, tok_], F32, kind="ExternalOutput")

    C = 2  # f-tiles per PSUM chunk (2 tiles x 2 banks = 4 banks in flight)
    KG = 4  # k-tiles per weight-slab DMA

    with tile.TileContext(nc) as tc, ExitStack() as ctx:
        xp = ctx.enter_context(tc.tile_pool(name="xp", bufs=KD))
        t1p = ctx.enter_context(tc.tile_pool(name="t1p", bufs=KH))
        t2p = ctx.enter_context(tc.tile_pool(name="t2p", bufs=KH))
        mp = ctx.enter_context(tc.tile_pool(name="mp", bufs=KD))
        wp = ctx.enter_context(tc.tile_pool(name="wp", bufs=3))
        wcp = ctx.enter_context(tc.tile_pool(name="wcp", bufs=2))
        bp = ctx.enter_context(tc.tile_pool(name="bp", bufs=2))
        ap = ctx.enter_context(tc.tile_pool(name="ap", bufs=2))
        psp = ctx.enter_context(
            tc.tile_pool(name="psp", bufs=3, space=bass.MemorySpace.PSUM)
        )
        pxp = ctx.enter_context(
            tc.tile_pool(name="pxp", bufs=1, space=bass.MemorySpace.PSUM)
        )

        # Input activations, feature-major, resident for the whole kernel.
        xt = []
        for k in range(KD):
            t = xp.tile([128, tok_], BF16, tag="x", name=f"x{k}")
            nc.sync.dma_start(t[:], xT[k * 128 : (k + 1) * 128, :])
            xt.append(t)

        # f32 accumulator for main, resident.
        macc = [mp.tile([128, tok_], F32, tag="macc", name=f"macc{d}") for d in range(KD)]

        def linear_phase(src, wd, ft, kt, out_cb, tag):
            """out[f] = sum_k wd[k-tile, f-tile].T @ src[k], f in 0..ft.

            wd is the DRAM weight view [kt*128, ft*128]. PSUM chunks of C
            f-tiles; weight slabs of KG k-tiles x C*128 columns stream
            through `wp`. out_cb(f, ps) consumes the accumulated [128,S,512]
            PSUM tile.
            """
            for c0 in range(0, ft, C):
                cf = min(C, ft - c0)
                ps = [
                    psp.tile([128, S, NSUB], F32, tag="ps", name=f"ps_{tag}_{c0}_{i}")
                    for i in range(cf)
                ]
                for kb in range(0, kt, KG):
                    kg = min(KG, kt - kb)
                    wt = wp.tile([128, kg, cf * 128], BF16, tag="w", name=f"w_{tag}_{c0}_{kb}")
                    nc.sync.dma_start(
                        wt[:],
                        wd[
                            kb * 128 : (kb + kg) * 128, c0 * 128 : (c0 + cf) * 128
                        ].rearrange("(kk p) j -> p kk j", p=128),
                    )
                    for kk in range(kg):
                        k = kb + kk
                        for i in range(cf):
                            for s in range(S):
                                nc.tensor.matmul(
                                    ps[i][:, s, :],
                                    wt[:, kk, i * 128 : (i + 1) * 128],
                                    src[k][:, s * NSUB : (s + 1) * NSUB],
                                    start=(k == 0),
                                    stop=(k == kt - 1),
                                )
                for i in range(cf):
                    out_cb(c0 + i, ps[i])

        for e in range(e_):
            act = act_fns[e]

            b1t = bp.tile([128, KH], F32, tag="b1", name=f"b1_{e}")
            nc.sync.dma_start(b1t[:], b1T[e])
            b2t = bp.tile([128, KH], F32, tag="b2", name=f"b2_{e}")
            nc.sync.dma_start(b2t[:], b2T[e])
            bct = bp.tile([out_, 1], F32, tag="bc", name=f"bc_{e}")
            nc.sync.dma_start(bct[:], bc[e])

            # ---- Phase 1: t1 = act(x @ W1 + b1), feature-major bf16
            t1 = []

            def p1_out(f, ps, e=e, act=act, b1t=b1t, t1=t1):
                tt = t1p.tile([128, tok_], BF16, tag="t1", name=f"t1_{e}_{f}")
                for s in range(S):
                    nc.scalar.activation(
                        tt[:, s * NSUB : (s + 1) * NSUB],
                        ps[:, s, :],
                        act,
                        bias=b1t[:, f : f + 1],
                    )
                t1.append(tt)

            linear_phase(xt, W1[e], KH, KD, p1_out, f"p1e{e}")

            # ---- Phase 2: t2 = act(t1 @ W2 + b2)
            t2 = []

            def p2_out(f, ps, e=e, act=act, b2t=b2t, t2=t2):
                tt = t2p.tile([128, tok_], BF16, tag="t2", name=f"t2_{e}_{f}")
                for s in range(S):
                    nc.scalar.activation(
                        tt[:, s * NSUB : (s + 1) * NSUB],
                        ps[:, s, :],
                        act,
                        bias=b2t[:, f : f + 1],
                    )
                t2.append(tt)

            linear_phase(t1, W2[e], KH, KH, p2_out, f"p2e{e}")

            # ---- Phase 3: macc += t2 @ Wm   (bias folded into bmsT at the end)
            def p3_out(dtile, ps, e=e):
                for s in range(S):
                    dst = macc[dtile][:, s * NSUB : (s + 1) * NSUB]
                    if e == 0:
                        nc.vector.tensor_copy(dst, ps[:, s, :])
                    else:
                        nc.vector.tensor_add(dst, dst, ps[:, s, :])

            linear_phase(t2, Wm[e], KD, KH, p3_out, f"p3e{e}")

            # ---- Phase 4: aux[e] = t2 @ Wc + bc
            wct = wcp.tile([128, KH, out_], BF16, tag="wc", name=f"wc_{e}")
            nc.sync.dma_start(wct[:], Wc[e].rearrange("(kk p) j -> p kk j", p=128))
            psa = pxp.tile([out_, S, NSUB], F32, tag="psa", name=f"psa_{e}")
            for k in range(KH):
                for s in range(S):
                    nc.tensor.matmul(
                        psa[:, s, :],
                        wct[:, k, :],
                        t2[k][:, s * NSUB : (s + 1) * NSUB],
                        start=(k == 0),
                        stop=(k == KH - 1),
                    )
            for s in range(S):
                ao = ap.tile([out_, NSUB], F32, tag="ao", name=f"ao_{e}_{s}")
                nc.scalar.activation(ao[:], psa[:, s, :], AF.Identity, bias=bct[:])
                nc.sync.dma_start(auxT[e][:, s * NSUB : (s + 1) * NSUB], ao[:])

        # ---- Epilogue: add summed main bias, store main.
        bmst = bp.tile([128, KD], F32, tag="bms", name="bms")
        nc.sync.dma_start(bmst[:], bmsT[:])
        for dtile in range(KD):
            nc.vector.tensor_scalar_add(
                macc[dtile][:], macc[dtile][:], bmst[:, dtile : dtile + 1]
            )
            nc.sync.dma_start(mainT[dtile * 128 : (dtile + 1) * 128, :], macc[dtile][:])

    nc.compile()
    return nc


def build_nc_min(e_=E, d_=D, h_=H, out_=OUT, tok_=TOK):
    """Minimal kernel with the identical I/O signature: touches every input
    with a tiny DMA and writes a tiny piece of each output. Used to measure
    the per-execution dispatch/binding overhead floor for differential
    timing."""
    KD, KH = d_ // 128, h_ // 128
    nc = bacc.Bacc(None, target_bir_lowering=False)
    xT = nc.dram_tensor("xT", [d_, tok_], BF16, kind="ExternalInput")
    W1 = nc.dram_tensor("W1", [e_, d_, h_], BF16, kind="ExternalInput")
    W2 = nc.dram_tensor("W2", [e_, h_, h_], BF16, kind="ExternalInput")
    Wm = nc.dram_tensor("Wm", [e_, h_, d_], BF16, kind="ExternalInput")
    Wc = nc.dram_tensor("Wc", [e_, h_, out_], BF16, kind="ExternalInput")
    b1T = nc.dram_tensor("b1T", [e_, 128, KH], F32, kind="ExternalInput")
    b2T = nc.dram_tensor("b2T", [e_, 128, KH], F32, kind="ExternalInput")
    bmsT = nc.dram_tensor("bmsT", [128, KD], F32, kind="ExternalInput")
    bc = nc.dram_tensor("bc", [e_, out_, 1], F32, kind="ExternalInput")
    mainT = nc.dram_tensor("mainT", [d_, tok_], F32, kind="ExternalOutput")
    auxT = nc.dram_tensor("auxT", [e_, out_, tok_], F32, kind="ExternalOutput")
    with tile.TileContext(nc) as tc, ExitStack() as ctx:
        p = ctx.enter_context(tc.tile_pool(name="p", bufs=2))
        for i, (src, n) in enumerate(
            [
                (xT[0:128, 0:8], 8),
                (W1[0, 0:128, 0:8], 8),
                (W2[0, 0:128, 0:8], 8),
                (Wm[0, 0:128, 0:8], 8),
                (Wc[0, 0:128, 0:4], 4),
                (b1T[0, :, 0:1], 1),
                (b2T[0, :, 0:1], 1),
                (bmsT[:, 0:1], 1),
            ]
        ):
            t = p.tile([128, n], src.dtype, tag=f"t{i}", name=f"t{i}")
            nc.sync.dma_start(t[:], src)
        tb = p.tile([out_, 1], F32, tag="tb", name="tb")
        nc.sync.dma_start(tb[:], bc[0])
        to = p.tile([128, 8], F32, tag="to", name="to")
        nc.vector.memset(to[:], 0.0)
        nc.sync.dma_start(mainT[0:128, 0:8], to[:])
        ta = p.tile([out_, 8], F32, tag="ta", name="ta")
        nc.vector.memset(ta[:], 0.0)
        nc.sync.dma_start(auxT[0][:, 0:8], ta[:])
    nc.compile()
    return nc


class _Exec:
    """Mirror of bass2jax.run_bass_via_pjrt's multi-core path, but keeps the
    jitted executable and the (large, replicated-weight) device inputs alive so
    repeated runs only move the small donated output buffers."""

    def __init__(self, nc, n_cores):
        import jax
        from jax.experimental.shard_map import shard_map
        from jax.sharding import Mesh, NamedSharding, PartitionSpec
        from concourse.bass2jax import (
            _bass_exec_p,
            install_neuronx_cc_hook,
            partition_id_tensor,
        )

        install_neuronx_cc_hook()
        assert nc.dbg_addr is None
        self.jax = jax
        self.nc = nc
        self.n_cores = n_cores
        partition_name = (
            nc.partition_id_tensor.name if nc.partition_id_tensor else None
        )
        in_names, out_names, out_avals, zero_outs = [], [], [], []
        for alloc in nc.m.functions[0].allocations:
            if not isinstance(alloc, mybir.MemoryLocationSet):
                continue
            name = alloc.memorylocations[0].name
            if alloc.kind == "ExternalInput":
                if name != partition_name:
                    in_names.append(name)
            elif alloc.kind == "ExternalOutput":
                out_names.append(name)
                shape = tuple(alloc.tensor_shape)
                dtype = mybir.dt.np(alloc.dtype)
                out_avals.append(jax.core.ShapedArray(shape, dtype))
                zero_outs.append(np.zeros(shape, dtype))
        n_params = len(in_names)
        in_names = in_names + out_names
        if partition_name is not None:
            in_names.append(partition_name)
        self.param_names = in_names[:n_params]
        self.out_names = out_names
        self.out_avals = out_avals
        self.zero_outs = zero_outs
        self.n_params = n_params

        def _bind(operands):
            return _bass_exec_p.bind(
                *operands,
                out_avals=tuple(out_avals),
                in_names=tuple(in_names),
                out_names=tuple(out_names),
                lowering_input_output_aliases=(),
                sim_require_finite=True,
                sim_require_nnan=True,
                nc=nc,
            )

        self._bind = _bind
        self._partition_id_tensor = partition_id_tensor
        self._partition_name = partition_name

        def _body(*args):
            operands = list(args)
            if partition_name is not None:
                operands.append(partition_id_tensor())
            return tuple(_bind(operands))

        devices = jax.devices()[:n_cores]
        assert len(devices) == n_cores
        self.mesh = Mesh(np.asarray(devices), ("core",))
        self.sharding = NamedSharding(self.mesh, PartitionSpec("core"))
        n_outs = len(out_names)
        donate = tuple(range(n_params, n_params + n_outs))
        in_specs = (PartitionSpec("core"),) * (n_params + n_outs)
        out_specs = (PartitionSpec("core"),) * n_outs
        self.sharded = jax.jit(
            shard_map(
                _body,
                mesh=self.mesh,
                in_specs=in_specs,
                out_specs=out_specs,
                check_rep=False,
            ),
            donate_argnums=donate,
            keep_unused=True,
        )
        self.in_dev = None

    def prepare(self, in_maps):
        """Concat per-core inputs along axis 0 and push to devices."""
        assert len(in_maps) == self.n_cores
        concat_in = [
            np.concatenate([np.asarray(m[name]) for m in in_maps], axis=0)
            for name in self.param_names
        ]
        self.in_dev = [
            self.jax.device_put(a, self.sharding) for a in concat_in
        ]
        self.jax.block_until_ready(self.in_dev)

    def run_once(self):
        """One full execution; returns (per_core_results, exec_seconds).

        The donated output buffers are staged to device before the timed
        region, so exec_seconds ≈ dispatch + NEFF execution."""
        zeros_dev = [
            self.jax.device_put(
                np.zeros((self.n_cores * z.shape[0], *z.shape[1:]), z.dtype),
                self.sharding,
            )
            for z in self.zero_outs
        ]
        self.jax.block_until_ready(zeros_dev)
        t0 = time.perf_counter()
        out_arrs = self.sharded(*self.in_dev, *zeros_dev)
        self.jax.block_until_ready(out_arrs)
        dt = time.perf_counter() - t0
        results = [
            {
                name: np.asarray(out_arrs[i]).reshape(
                    self.n_cores, *self.out_avals[i].shape
                )[c]
                for i, name in enumerate(self.out_names)
            }
            for c in range(self.n_cores)
        ]
        return results, dt

    def bench_pipelined(self, k=10, rounds=3):
        """Dispatch k executions without blocking in between, block once at
        the end; returns best seconds/execution over `rounds`. Per-call RPC
        overhead is pipelined against device execution, so this approaches
        (per-call dispatch throughput + device exec). Subtract the same
        measurement of a minimal same-I/O kernel to isolate device exec."""

        def zero_sets():
            zs = [
                [
                    self.jax.device_put(
                        np.zeros((self.n_cores * z.shape[0], *z.shape[1:]), z.dtype),
                        self.sharding,
                    )
                    for z in self.zero_outs
                ]
                for _ in range(k)
            ]
            self.jax.block_until_ready(zs)
            return zs

        best = None
        for _ in range(rounds):
            zs = zero_sets()
            t0 = time.perf_counter()
            outs = [self.sharded(*self.in_dev, *zs[i]) for i in range(k)]
            self.jax.block_until_ready(outs)
            dt = (time.perf_counter() - t0) / k
            best = dt if best is None else min(best, dt)
        return best


_CACHE = {}


def get_exec():
    if "exec" not in _CACHE:
        _CACHE["exec"] = _Exec(build_nc(), N_CORES)
    return _CACHE["exec"]


def prep_inputs(h, W1, b1, W2, b2, Wm, bm, Wc, bc, n_cores=N_CORES):
    """Host-side shard + layout prep. Returns per-core input maps."""
    e_, d_, h_ = W1.shape
    out_ = Wc.shape[2]
    bt = h.shape[0] * h.shape[1]
    tok = bt // n_cores
    h_flat = np.ascontiguousarray(h.reshape(bt, d_))

    w1b = np.ascontiguousarray(W1.astype(NPBF16))
    w2b = np.ascontiguousarray(W2.astype(NPBF16))
    wmb = np.ascontiguousarray(Wm.astype(NPBF16))
    wcb = np.ascontiguousarray(Wc.astype(NPBF16))
    b1t = np.ascontiguousarray(
        b1.reshape(e_, h_ // 128, 128).transpose(0, 2, 1).astype(np.float32)
    )
    b2t = np.ascontiguousarray(
        b2.reshape(e_, h_ // 128, 128).transpose(0, 2, 1).astype(np.float32)
    )
    bmst = np.ascontiguousarray(
        bm.sum(axis=0).reshape(d_ // 128, 128).T.astype(np.float32)
    )
    bcr = np.ascontiguousarray(bc.reshape(e_, out_, 1).astype(np.float32))

    in_maps = []
    for c in range(n_cores):
        xTc = np.ascontiguousarray(
            h_flat[c * tok : (c + 1) * tok, :].T.astype(NPBF16)
        )
        in_maps.append(
            {
                "xT": xTc,
                "W1": w1b,
                "W2": w2b,
                "Wm": wmb,
                "Wc": wcb,
                "b1T": b1t,
                "b2T": b2t,
                "bmsT": bmst,
                "bc": bcr,
            }
        )
    return in_maps


def assemble_outputs(results, b_, t_, e_, d_, out_):
    """Gather per-core feature-major outputs back to reference layout."""
    main = np.concatenate([np.asarray(r["mainT"]).T for r in results], axis=0)
    main = np.ascontiguousarray(main.reshape(b_, t_, d_).astype(np.float32))
    aux = np.concatenate(
        [np.asarray(r["auxT"]).transpose(0, 2, 1) for r in results], axis=1
    )
    aux = np.ascontiguousarray(aux.reshape(e_, b_, t_, out_).astype(np.float32))
    return main, aux


def run(inputs):
    """Run on hardware; returns ((main, aux), exec_seconds)."""
    ex = get_exec()
    inputs = {k: np.asarray(v) for k, v in inputs.items()}
    in_maps = prep_inputs(**inputs)
    ex.prepare(in_maps)
    results, dt = ex.run_once()
    main, aux = assemble_outputs(results, B, T, E, D, OUT)
    return (main, aux), dt


def kernel(**inputs):
    out, _ = run(inputs)
    return out
